# revision 19
# baseline (speedup 1.0000x reference)
"""Trainium2 Bass kernel for nn_ConditionedDense (hypernetwork-conditioned dense).

Reference computation:
    A = einsum('bnp,pq->bnq', P, Wk)         # hypernetwork: per-position weights
    W = relu(A).reshape(B, N, c_in, c_out)
    out = einsum('bni,bnio->bno', X, W)

Strategy: pure data parallel over 8 NeuronCores (shard batch dim), 16384
positions/core, all in a transposed layout [q, pos] (q = o*32+i) processed
as 32 supers of 512 positions:

 - PE: per super, 8 hypernet matmuls (lhsT = Wk q-block [64,128], rhs =
   P^T [64,512]) fill A^T tiles [128 q, 512 pos] in PSUM, then 8
   accumulating reduce-matmuls (lhsT = 0/1 selector S_blk [128,32], rhs =
   m-block) compute out^T[32 o, 512] in a PSUM bank.  The reduce matmuls
   are emitted one super late so the in-order PE queue never stalls on DVE.
 - Evacuation (PSUM f32 -> SBUF bf16 + relu) of the 16.8M A elements is
   the scarce resource: only ACT and DVE can read PSUM (GPSIMD cannot,
   DMA has no PSUM route).  Units of [128, 1024] split ACT:DVE = 2:1.
 - multiply W^T by the 4x-partition-replicated X^T (bf16 tensor_tensor,
   2x mode): on DVE for 3 of 4 supers, on GPSIMD (slow but otherwise
   idle) every 4th super.

Engine budget/core: PE ~109us, ACT ~109, DVE ~105, GPSIMD ~66
(baseline: 177us, DVE-bound).  PE is ~100% occupied which also keeps it
out of the slow DVFS p-state.
Host side (free): shard, transpose P, permute Wk to q=o*32+i, build the
replicated X^T and selectors S, cast to bf16; un-transpose outputs.
"""

import os
from contextlib import ExitStack

import numpy as np
import ml_dtypes

import concourse.bass as bass
import concourse.tile as tile
from concourse import bacc, mybir
from concourse.bass_utils import run_bass_kernel_spmd

C_IN = 32
C_OUT = 32
P_DIM = 64
Q = C_IN * C_OUT  # 1024
B, N = 32, 4096
N_CORES = 8
B_SH = B // N_CORES          # 4 batches per core
NPOS = B_SH * N              # 16384 positions per core
TILE_P = 128

# A-path (DVE-tree) chunks of 1024 positions, interleaved among B supers
A_CHUNKS = 4
A_TPC = 8                    # position-tiles per A chunk
A_POS = A_CHUNKS * A_TPC * TILE_P      # 4096
B_SUP = 512                  # positions per super (B path)
N_SUP = (NPOS - A_POS) // B_SUP        # 24
OUT_GRP = 2                  # supers per [64, 512] PSUM out container
N_OGRP = N_SUP // OUT_GRP    # 12

F32 = mybir.dt.float32
BF16 = mybir.dt.bfloat16

_BUILD_CACHE = {}
LAST_RESULTS = None  # BassKernelResults of the most recent run (for profiling)

# Evacuation engine pattern: True -> ACT, False -> DVE (3:1).
EVAC_PAT = [True, True, False, True]
# of the 24 B-super multiplies, run these on GPSIMD instead of DVE (13/24)
POOL_MUL = set(range(0, 24, 2)) | {23}
# reduce matmuls for super s are emitted SKEW B-steps later so the in-order
# PE queue never waits on a (possibly slow GPSIMD) multiply
SKEW = 3


def _build_nc():
    nc = bacc.Bacc(
        "TRN2", target_bir_lowering=False, debug=False, num_devices=N_CORES
    )
    PT_d = nc.declare_dram_parameter("PT", [P_DIM, NPOS], BF16, isOutput=False)
    Wk_d = nc.declare_dram_parameter("Wk", [P_DIM, Q], BF16, isOutput=False)
    X_d = nc.declare_dram_parameter("X", [A_POS, C_IN], BF16, isOutput=False)
    # XR: X^T replicated 4x along partitions, B positions only
    XR_d = nc.declare_dram_parameter(
        "XR", [TILE_P, NPOS - A_POS], BF16, isOutput=False
    )
    # S: per-block reduce selectors, [128, 8*32] bf16
    S_d = nc.declare_dram_parameter("S", [TILE_P, 8 * C_OUT], BF16, isOutput=False)
    outA_d = nc.declare_dram_parameter("outA", [A_POS, C_OUT], BF16, isOutput=True)
    # outB: [ogrp, 64, 512] where row r = 32*(sup%2) + o
    out_d = nc.declare_dram_parameter(
        "outB", [N_OGRP, 2 * C_OUT, B_SUP], BF16, isOutput=True
    )

    relu = mybir.ActivationFunctionType.Relu
    mult = mybir.AluOpType.mult
    add = mybir.AluOpType.add

    ev_idx = [0]  # round-robin state for evac engine assignment

    with ExitStack() as ctx:
        tc = ctx.enter_context(tile.TileContext(nc))
        wkp = ctx.enter_context(tc.tile_pool(name="wk", bufs=1))
        sp_ = ctx.enter_context(tc.tile_pool(name="sel", bufs=1))
        pp = ctx.enter_context(tc.tile_pool(name="pT", bufs=4))
        xrp = ctx.enter_context(tc.tile_pool(name="xr", bufs=4))
        # PSUM: A^T blk-pair tiles [128, 2, 512] f32 (2 banks) x3 rotation
        apool = ctx.enter_context(tc.tile_pool(name="apsum", bufs=3, space="PSUM"))
        # PSUM: out accumulator [64, 512] f32 (1 bank) x2
        opsum = ctx.enter_context(tc.tile_pool(name="opsum", bufs=2, space="PSUM"))
        wbp = ctx.enter_context(tc.tile_pool(name="wb", bufs=4))
        mbp = ctx.enter_context(tc.tile_pool(name="mb", bufs=5))
        obp = ctx.enter_context(tc.tile_pool(name="ob", bufs=3))
        xp = ctx.enter_context(tc.tile_pool(name="x", bufs=2))
        wp = ctx.enter_context(tc.tile_pool(name="w", bufs=2))
        mp = ctx.enter_context(tc.tile_pool(name="m", bufs=2))
        t1p = ctx.enter_context(tc.tile_pool(name="t1", bufs=2))
        t2p = ctx.enter_context(tc.tile_pool(name="t2", bufs=2))
        t3p = ctx.enter_context(tc.tile_pool(name="t3", bufs=2))
        t4p = ctx.enter_context(tc.tile_pool(name="t4", bufs=2))
        op = ctx.enter_context(tc.tile_pool(name="o", bufs=2))

        wk_t = wkp.tile([P_DIM, Q], BF16)
        nc.sync.dma_start(out=wk_t[:], in_=Wk_d[:])
        s_t = sp_.tile([TILE_P, 8 * C_OUT], BF16)
        nc.sync.dma_start(out=s_t[:], in_=S_d[:])

        def evac(dst, src):
            """PSUM f32 -> SBUF bf16 with relu, alternating ACT / DVE."""
            if EVAC_PAT[ev_idx[0] % len(EVAC_PAT)]:
                nc.scalar.activation(dst, src, relu)
            else:
                nc.vector.tensor_scalar_max(dst, src, 0.0)
            ev_idx[0] += 1

        pending = []     # [(m_b tile, super index)] awaiting reduce matmuls
        ogrp_ps = None   # current [64, 512] PSUM out container
        ogrp_n = 0
        post = []        # [(due_step, closure)] deferred emissions
        step = [0]

        def flush_post():
            rest = []
            for due, f in post:
                if due <= step[0]:
                    f()
                else:
                    rest.append((due, f))
            post[:] = rest

        def emit_reduce(pend):
            nonlocal ogrp_ps, ogrp_n
            m_b, s = pend
            if ogrp_ps is None:
                ogrp_ps = opsum.tile([2 * C_OUT, B_SUP], F32)
                ogrp_n = 0
            row = 32 * (s % OUT_GRP)
            dst = ogrp_ps[row:row + 32, :]
            for blk in range(8):
                nc.tensor.matmul(
                    dst, lhsT=s_t[:, bass.ts(blk, C_OUT)], rhs=m_b[:, blk, :],
                    start=(blk == 0), stop=(blk == 7), skip_group_check=True,
                )
            ogrp_n += 1
            g = s // OUT_GRP
            if ogrp_n == OUT_GRP or s == N_SUP - 1:
                ps = ogrp_ps

                def copy_out(ps=ps, g=g):
                    ob = obp.tile([2 * C_OUT, B_SUP], BF16)
                    nc.scalar.activation(
                        ob[:], ps[:], mybir.ActivationFunctionType.Copy
                    )
                    post.append((step[0] + 1, lambda: nc.sync.dma_start(
                        out=out_d[g, :, :], in_=ob[:]
                    )))

                post.append((step[0] + 1, copy_out))
                ogrp_ps = None

        def emit_a_chunk(ch):
            # A-path: [pos, q] layout, DVE mul + halving tree over i
            pos0 = ch * A_TPC * TILE_P
            x_c = xp.tile([TILE_P, A_TPC, C_IN], BF16)
            nc.sync.dma_start(
                out=x_c[:],
                in_=X_d[pos0:pos0 + A_TPC * TILE_P, :].rearrange(
                    "(a p) i -> p a i", p=TILE_P
                ),
            )
            pT_c = pp.tile([P_DIM, A_TPC * TILE_P], BF16)
            nc.sync.dma_start(
                out=pT_c[:], in_=PT_d[:, pos0:pos0 + A_TPC * TILE_P]
            )
            w_c = wp.tile([TILE_P, A_TPC, Q], BF16)
            for h in range(A_TPC):
                a_t = apool.tile([TILE_P, Q], F32)
                lhsT = pT_c[:, bass.ts(h, TILE_P)]
                nc.tensor.matmul(
                    a_t[:, 0:512], lhsT=lhsT, rhs=wk_t[:, 0:512],
                    start=True, stop=True, skip_group_check=True,
                )
                nc.tensor.matmul(
                    a_t[:, 512:1024], lhsT=lhsT, rhs=wk_t[:, 512:1024],
                    start=True, stop=True, skip_group_check=True,
                )
                evac(w_c[:, h, :], a_t[:])

            m_t = mp.tile([TILE_P, A_TPC, Q], BF16)
            w4 = w_c[:].rearrange("p j (o i) -> p j o i", o=C_OUT)
            m4 = m_t[:].rearrange("p j (o i) -> p j o i", o=C_OUT)
            x4 = x_c[:].unsqueeze(2).broadcast_to(
                [TILE_P, A_TPC, C_OUT, C_IN]
            )
            nc.vector.tensor_tensor(out=m4, in0=w4, in1=x4, op=mult)
            t1 = t1p.tile([TILE_P, A_TPC, C_OUT, 16], BF16)
            nc.vector.tensor_tensor(
                out=t1[:], in0=m4[:, :, :, 0:16], in1=m4[:, :, :, 16:32],
                op=add,
            )
            t2 = t2p.tile([TILE_P, A_TPC, C_OUT, 8], BF16)
            nc.vector.tensor_tensor(
                out=t2[:], in0=t1[:, :, :, 0:8], in1=t1[:, :, :, 8:16], op=add,
            )
            t3 = t3p.tile([TILE_P, A_TPC, C_OUT, 4], BF16)
            nc.vector.tensor_tensor(
                out=t3[:], in0=t2[:, :, :, 0:4], in1=t2[:, :, :, 4:8], op=add,
            )
            t4 = t4p.tile([TILE_P, A_TPC, C_OUT, 2], BF16)
            nc.vector.tensor_tensor(
                out=t4[:], in0=t3[:, :, :, 0:2], in1=t3[:, :, :, 2:4], op=add,
            )
            o_c = op.tile([TILE_P, A_TPC, C_OUT], BF16)
            nc.vector.tensor_tensor(
                out=o_c[:].unsqueeze(3), in0=t4[:, :, :, 0:1],
                in1=t4[:, :, :, 1:2], op=add,
            )
            post.append((step[0] + 1, lambda: nc.sync.dma_start(
                out=outA_d[pos0:pos0 + A_TPC * TILE_P, :].rearrange(
                    "(a p) i -> p a i", p=TILE_P
                ),
                in_=o_c[:],
            )))

        # spread A chunks among B supers: one after B supers 4, 10, 16, 22
        a_after = {4: 0, 10: 1, 16: 2, 22: 3}
        bpair = None
        for s in range(N_SUP):
            step[0] = s
            flush_post()
            if s - 1 in a_after:
                emit_a_chunk(a_after[s - 1])
            pos0 = A_POS + s * B_SUP
            if s % 2 == 0:
                n2 = min(2, N_SUP - s) * B_SUP
                pT_pair = pp.tile([P_DIM, n2], BF16)
                xr_pair = xrp.tile([TILE_P, n2], BF16)
                bpair = (pT_pair, xr_pair)
                nc.sync.dma_start(out=pT_pair[:], in_=PT_d[:, pos0:pos0 + n2])
                nc.sync.dma_start(
                    out=xr_pair[:], in_=XR_d[:, s * B_SUP:s * B_SUP + n2]
                )
            off = (s % 2) * B_SUP
            pT_s = bpair[0][:, off:off + B_SUP]
            xr_s = bpair[1][:, off:off + B_SUP]

            w_b = wbp.tile([TILE_P, 8, B_SUP], BF16)
            for bp in range(4):  # block pairs
                a_t = apool.tile([TILE_P, 2, B_SUP], F32)
                for j in range(2):
                    blk = bp * 2 + j
                    nc.tensor.matmul(
                        a_t[:, j, :], lhsT=wk_t[:, bass.ts(blk, TILE_P)],
                        rhs=pT_s, start=True, stop=True,
                        skip_group_check=True,
                    )
                evac(w_b[:, bp * 2:bp * 2 + 2, :], a_t[:])

            # m = w * xr (xr broadcast over the 8 q-blocks)
            m_b = mbp.tile([TILE_P, 8, B_SUP], BF16)
            xin = xr_s.unsqueeze(1).broadcast_to([TILE_P, 8, B_SUP])
            if s in POOL_MUL:
                nc.gpsimd.tensor_tensor(out=m_b[:], in0=w_b[:], in1=xin, op=mult)
            else:
                nc.vector.tensor_tensor(out=m_b[:], in0=w_b[:], in1=xin, op=mult)

            pending.append((m_b, s))
            if len(pending) > SKEW:
                emit_reduce(pending.pop(0))

        for pend in pending:
            emit_reduce(pend)
        while post:
            step[0] += 1
            flush_post()

    nc.finalize()
    return nc


def _get_nc():
    key = "v9"
    if key not in _BUILD_CACHE:
        _BUILD_CACHE[key] = _build_nc()
    return _BUILD_CACHE[key]


def kernel(X, P, Wk):
    global LAST_RESULTS
    X = np.asarray(X, dtype=np.float32)
    P = np.asarray(P, dtype=np.float32)
    Wk = np.asarray(Wk, dtype=np.float32)
    bf16 = ml_dtypes.bfloat16

    # Host-side prep (free): shard, transpose P, permute Wk columns so the
    # device-side layout is q = o*32+i; cast matmul operands to bf16.
    WkP = np.ascontiguousarray(
        Wk.reshape(P_DIM, C_IN, C_OUT).transpose(0, 2, 1).reshape(P_DIM, Q)
    ).astype(bf16)
    # Selector S[r, blk*32 + o] = 1 iff o == 4*blk + r//32  (q = o*32+i)
    S = np.zeros((TILE_P, 8 * C_OUT), dtype=np.float32)
    r = np.arange(TILE_P)
    for blk in range(8):
        S[r, blk * C_OUT + 4 * blk + r // 32] = 1.0
    S = S.astype(bf16)

    in_maps = []
    for c in range(N_CORES):
        Xc = X[c * B_SH:(c + 1) * B_SH].reshape(NPOS, C_IN)
        PTc = np.ascontiguousarray(
            P[c * B_SH:(c + 1) * B_SH].reshape(NPOS, P_DIM).T
        ).astype(bf16)
        XA = np.ascontiguousarray(Xc[:A_POS]).astype(bf16)
        # XR[r, j] = X[A_POS + j, r % 32]  (X^T tiled 4x along partitions)
        XRc = np.ascontiguousarray(np.tile(Xc[A_POS:].T, (4, 1))).astype(bf16)
        in_maps.append({"PT": PTc, "Wk": WkP, "X": XA, "XR": XRc, "S": S})

    nc = _get_nc()
    trace = os.environ.get("BASS_PROFILE", "0") == "1"
    kw = {}
    if os.environ.get("BASS_TMPDIR"):
        kw["tmpdir"] = os.environ["BASS_TMPDIR"]
    res = run_bass_kernel_spmd(
        nc, in_maps, list(range(N_CORES)), trace=trace, **kw
    )
    LAST_RESULTS = res

    out = np.empty((B, N, C_OUT), dtype=np.float32)
    for c in range(N_CORES):
        sh = out[c * B_SH:(c + 1) * B_SH].reshape(NPOS, C_OUT)
        sh[:A_POS] = np.asarray(res.results[c]["outA"]).astype(np.float32)
        ob = np.asarray(res.results[c]["outB"]).astype(np.float32)
        # ob[g, 32*(s%2)+o, p] -> position A_POS + s*512 + p
        for s in range(N_SUP):
            g, rr = divmod(s, OUT_GRP)
            sh[A_POS + s * B_SUP:A_POS + (s + 1) * B_SUP] = (
                ob[g, rr * 32:(rr + 1) * 32, :].T
            )
    return out
